# revision 2
# baseline (speedup 1.0000x reference)
"""Trainium2 Bass kernel for the ContrastiveLoss problem.

Reference semantics (N=M=8192, D=512, C=1000):
    valid = labels1 > 0 ; n = sum(valid)
    sim   = inputs1 @ inputs2.T                       # [N, M]
    same  = labels1[:, None] == labels2[None, :]
    pos_sel = same  & (sim < 1 - EPS - POS_MARGIN) & valid[:, None]
    neg_sel = ~same & (sim > MARGIN)               & valid[:, None]
    loss = (sum(1-sim | pos_sel) + sum(sim | neg_sel)) / n
    avg_neg = count(neg_sel) / n
    avg_pos = round(100 * count(pos_sel) / n) / 100

Strategy (8 NeuronCores, data-parallel over rows of inputs1):
  * Host folds the row-validity mask into the operands (x1 row := 0,
    label := -1), so the device needs no validity logic.
  * Each core computes its [1024, 8192] slice of sim as fp8e4m3
    DoubleRow matmuls (fp32 PSUM accumulation). Host pre-interleaves
    both operands as [partition, chunk, pair, cols].
  * Per PSUM tile ([128, 1024], 2 banks, 4-deep rotation) ONE fused
    elementwise+row-reduce pass runs directly on PSUM:
    sum(relu(s - CERT_T)) per row, alternating between ScalarE and
    VectorE so both trail the PE comfortably. Nothing else leaves the
    device except these [128, 64] accumulator tiles.
  * CERT_T = 0.3125 is a *certificate* threshold: worst-case fp8e4m3
    input-quantization error on a unit-norm dot product is < 0.18, so
    any true sim > 0.5 implies fp8-sim > 0.32 > CERT_T and would make
    its accumulator slot nonzero. All slots == 0 therefore PROVES
    neg_sel is empty and every same-label sim < 0.95-eps, i.e.
    pos_sel == same & valid.
  * Host then finishes exactly with label-bucket algebra:
    sum(sim | same & valid) = sum_c A_c . B_c  where A_c/B_c are the
    per-label sums of x1(masked)/x2 rows -- O(N*D), fp64.
  * If a slot IS nonzero (never for this data distribution), the host
    recomputes exactly the flagged [row, 1024-col band] blocks in fp64
    and applies per-pair corrections. Bounded, exact fallback.
"""

import numpy as np
import ml_dtypes

N, M, D = 8192, 8192, 512
NCORES = 8
ROWS = N // NCORES  # rows of inputs1 per core
MARGIN = 0.5
POS_MARGIN = 0.05
EPS = 1e-6

# Certificate threshold: relu(sim_fp8 - CERT_T) summed per (row, band).
# 0.3125 + 0.18 (worst-case fp8 dot error at ||x||=1) < 0.5, and the
# actual max |sim| for this distribution is ~0.27+-0.01 << 0.3125.
CERT_T = 0.3125

BAND = 1024          # columns per PSUM tile (2 banks)
NB = M // BAND       # 8 column bands
MT = ROWS // 128     # 8 row tiles per core
NTILES = NB * MT     # 64 tiles -> 64 accumulator slots


def _on_act(idx: int) -> bool:
    """Engine assignment per tile: ~34 on ScalarE, ~30 on VectorE,
    strictly alternating (mod the period-15 wrap) so each engine sees
    a 2-tile issue interval."""
    return (idx % 15) % 2 == 0


_NC = None


def _build_program():
    import concourse.tile as tile
    from concourse import bacc, mybir

    nc = bacc.Bacc(
        "TRN2", target_bir_lowering=False, debug=False, num_devices=NCORES
    )
    bf16 = mybir.dt.bfloat16
    f32 = mybir.dt.float32
    fp8 = mybir.dt.float8e4

    # host pre-arranges inputs as [p(128), chunk(2), pair(2), cols]
    x1t = nc.dram_tensor("x1t", [128, 4 * ROWS], fp8, kind="ExternalInput").ap()
    x2t = nc.dram_tensor("x2t", [128, 4 * M], fp8, kind="ExternalInput").ap()
    stats_r = nc.dram_tensor("stats_r", [128, NTILES], f32, kind="ExternalOutput").ap()
    stats_a = nc.dram_tensor("stats_a", [128, NTILES], f32, kind="ExternalOutput").ap()

    with tile.TileContext(nc) as tc:
        with (
            tc.tile_pool(name="cbp", bufs=1) as cbp,
            tc.tile_pool(name="x1p", bufs=1) as x1p,
            tc.tile_pool(name="x2p", bufs=1) as x2p,
            tc.tile_pool(name="psp", bufs=4, space="PSUM") as psp,
            tc.tile_pool(name="scp", bufs=4) as scp,
            tc.tile_pool(name="stp", bufs=1) as stp,
        ):
            # const AP for the ScalarE Relu bias; the memset is a
            # tracked tile write, so no engine barrier is needed.
            cb = cbp.tile([128, 1], f32, tag="cb")
            nc.vector.memset(cb[:], -float(CERT_T))
            nc.const_aps.aps[(f32, -float(CERT_T))] = cb[:]

            # Input loads. Two HW DGE rings in parallel: sync carries
            # x2 bands 0-3, scalar carries x1 (needed first) + bands
            # 4-7. First matmul only waits for x1 + band 0 (1 MB).
            x1s = x1p.tile([128, 2, 2, ROWS], fp8)
            x1v = x1t.rearrange("p (c r m) -> p c r m", c=2, r=2)
            x2s = x2p.tile([128, 2, 2, M], fp8)
            x2v = x2t.rearrange("p (c r j) -> p c r j", c=2, r=2)
            nc.scalar.dma_start(x1s[:], x1v[:])
            for b in range(NB):
                eng = nc.sync if b < 4 else nc.scalar
                eng.dma_start(
                    x2s[:, :, :, b * BAND : (b + 1) * BAND],
                    x2v[:, :, :, b * BAND : (b + 1) * BAND],
                )

            stats_rt = stp.tile([128, NTILES], f32, tag="str")
            stats_at = stp.tile([128, NTILES], f32, tag="sta")

            for jb in range(NB):
                for m in range(MT):
                    idx = jb * MT + m
                    ps = psp.tile([128, BAND], f32)
                    # c-outer so each weight tile streams two matmuls.
                    for c in range(2):
                        for jj in range(2):
                            j0 = jb * BAND + jj * 512
                            nc.tensor.matmul(
                                ps[:, jj * 512 : (jj + 1) * 512],
                                x1s[:, c, :, m * 128 : (m + 1) * 128],
                                x2s[:, c, :, j0 : j0 + 512],
                                start=(c == 0),
                                stop=(c == 1),
                                perf_mode=mybir.MatmulPerfMode.DoubleRow,
                            )
                    scr = scp.tile([128, BAND], bf16, tag="scr")
                    if _on_act(idx):
                        nc.scalar.activation(
                            scr[:],
                            ps[:],
                            mybir.ActivationFunctionType.Relu,
                            bias=-float(CERT_T),
                            accum_out=stats_at[:, idx : idx + 1],
                        )
                    else:
                        nc.vector.tensor_scalar(
                            scr[:],
                            ps[:],
                            float(CERT_T),
                            0.0,
                            mybir.AluOpType.subtract,
                            mybir.AluOpType.max,
                            accum_out=stats_rt[:, idx : idx + 1],
                        )

            nc.sync.dma_start(stats_r[:], stats_rt[:])
            nc.sync.dma_start(stats_a[:], stats_at[:])

    nc.compile()
    return nc


def _get_program():
    global _NC
    if _NC is None:
        _NC = _build_program()
    return _NC


def _arrange(aT):  # [D, cols] fp8 -> [p(128), chunk*pair*cols]
    cols = aT.shape[1]
    return np.ascontiguousarray(
        aT.reshape(2, 2, 128, cols).transpose(2, 0, 1, 3).reshape(128, -1)
    )


def run(inputs, trace=False):
    from concourse.bass_utils import run_bass_kernel_spmd

    x1 = np.asarray(inputs["inputs1"], dtype=np.float32)
    l1 = np.asarray(inputs["labels1"]).astype(np.int64)
    x2 = np.asarray(inputs["inputs2"], dtype=np.float32)
    l2 = np.asarray(inputs["labels2"]).astype(np.int64)

    valid = l1 > 0
    n = int(valid.sum())

    # Fold the row-validity mask into the operands: sim rows of invalid
    # rows become 0 (-> no contribution anywhere).
    x1mf = np.where(valid[:, None], x1, np.float32(0))
    fp8 = ml_dtypes.float8_e4m3

    x1T = _arrange(x1mf.T.astype(fp8))
    x2T = _arrange(x2.T.astype(fp8))
    in_maps = [
        {
            "x1t": np.ascontiguousarray(
                x1T.reshape(128, 4, N)[:, :, c * ROWS : (c + 1) * ROWS].reshape(
                    128, -1
                )
            ),
            "x2t": x2T,
        }
        for c in range(NCORES)
    ]

    nc = _get_program()
    res = run_bass_kernel_spmd(nc, in_maps, core_ids=list(range(NCORES)), trace=trace)

    on_act = np.array([_on_act(i) for i in range(NTILES)])

    # --- exact same-label sums via label buckets (fp64) ---
    l1m = np.where(valid, l1, -1)
    nl = int(max(l1.max(), l2.max())) + 1
    x1d = np.where(valid[:, None], x1.astype(np.float64), 0.0)
    x2d = x2.astype(np.float64)
    A = np.zeros((nl, D), dtype=np.float64)
    np.add.at(A, l1m[valid], x1d[valid])
    B = np.zeros((nl, D), dtype=np.float64)
    np.add.at(B, l2, x2d)
    s_same = float((A * B).sum())
    c1 = np.bincount(l1m[valid], minlength=nl).astype(np.float64)
    c2 = np.bincount(l2, minlength=nl).astype(np.float64)
    pos_cnt = float(c1 @ c2)  # count(same & valid)

    pos_loss = pos_cnt - s_same
    neg_val = 0.0
    neg_cnt = 0.0

    # --- certificate check; exact per-block fallback if it fires ---
    pos_thresh = np.float32(1.0) - np.float32(EPS) - np.float32(POS_MARGIN)
    for c in range(NCORES):
        sr = res.results[c]["stats_r"].astype(np.float64)
        sa = res.results[c]["stats_a"].astype(np.float64)
        flags = np.zeros((128, NTILES), dtype=bool)
        flags[:, ~on_act] = sr[:, ~on_act] > 0
        flags[:, on_act] |= sa[:, on_act] > 0
        if not flags.any():
            continue
        for p, idx in zip(*np.nonzero(flags)):
            jb, m = divmod(int(idx), MT)
            row = c * ROWS + m * 128 + int(p)
            cols = slice(jb * BAND, (jb + 1) * BAND)
            s = x1d[row] @ x2d[cols.start : cols.stop].T  # exact, fp64
            same = l1m[row] == l2[cols]
            nm = (~same) & (s > MARGIN)
            neg_val += s[nm].sum()
            neg_cnt += int(nm.sum())
            pm = same & (s >= float(pos_thresh))
            if pm.any():
                pos_loss -= (1.0 - s[pm]).sum()
                pos_cnt -= int(pm.sum())

    loss = np.float32((pos_loss + neg_val) / n)
    avg_neg = np.float32(neg_cnt / n)
    avg_pos = np.float32(np.round(100.0 * pos_cnt / n) / 100.0)
    out = (
        np.array(loss, dtype=np.float32),
        np.array(avg_neg, dtype=np.float32),
        np.array(avg_pos, dtype=np.float32),
    )
    return out, res


def kernel(**inputs):
    out, _ = run(inputs)
    return out
